# revision 40
# baseline (speedup 1.0000x reference)
"""Trainium2 Bass kernel for 2D cubic Hermite interpolation (nn_CubicHermite2d).

Math: with x1 = arange(W), x2 = arange(H) (per the problem spec), the whole
op is linear in `signal`:

    result[b, r, q] = sum_{h,w} M2[h, r] * signal[b, h, w] * M1[w, q]

where M1 [W, Nx] / M2 [H, Ny] are 4-banded cubic-Hermite interpolation
matrices built on the host from xs / ys.  Queries are sorted, so greedy
contiguous query groups have source-row bands inside a single 128-row
window -> every output block is ONE K=128 matmul on the PE (no
accumulation, no transposes):

    step 1:  v[wlo][wp, r]  = sig[hlo:+128, wlo:+128].T @ M2[hlo:+128, rs:re]
    step 2:  out[b, rm, q]  = v[wlo][:, rm*128:+128].T @ M1[wlo:+128, qs:qe]

Matmuls default to float16: 1 cyc/row on the PE, FWL fast weight loads, and
half the load bytes; the inputs are O(1) randn so fp16 range is a non-issue
and the only cost is ~2^-12 input rounding (measured 9.1e-4 scale-relative
absmax vs the fp32 reference).  CH2D_DT=f32r/f32/bf16 selects other modes
(f32 is exact but 4 cyc/row).  Groups keep even sizes/starts so the f32r
mode also satisfies its even-N / 8B-aligned-PSUM ISA restrictions.

Each signal window loads for ALL batches in one 3D strided DMA; the build
software-pipelines step1(b+1) between the two step2 halves of batch b to
smooth store traffic, and the last batch stores per-r-block so the kernel
tail only drains a single 512KB store.

Sharding: data-parallel over batch B=32 across 8 cores (4 batches/core).
"""

import os
import sys

import numpy as np

for _p in ("/root/.axon_site", "/root/.axon_site/_ro/trn_rl_repo",
           "/root/.axon_site/_ro/pypackages", "/opt/trn_rl_repo"):
    if os.path.isdir(_p) and _p not in sys.path:
        sys.path.append(_p)

import concourse.bass as bass
import concourse.mybir as mybir
from concourse import bacc
from concourse.bass_utils import run_bass_kernel_spmd
from concourse.tile import TileContext

# Problem shapes (hardcoded per spec)
B, H, W = 32, 512, 512
NX, NY = 1024, 1024
N_CORES = 8
NB = B // N_CORES  # batches per core

P = 128
F32 = mybir.dt.float32
# matmul operand dtype: f16 (1 cyc/row, ~2^-12 input rounding, FWL weight
# loads, half DMA bytes) | f32r (2 cyc/row, ~2^-11 rounding) | f32 (4 cyc/row,
# exact) | bf16 (1 cyc/row, ~2^-9 rounding)
MM_MODE = os.environ.get("CH2D_DT", "f16")
_MM_DTS = {"f16": mybir.dt.float16, "bf16": mybir.dt.bfloat16,
           "f32r": mybir.dt.float32r, "f32": mybir.dt.float32}
# COARSE_COPY: one [128,1024] PSUM->SBUF copy per tile (2-bank PSUM tiles,
# fewer engine ops) vs two [128,512] copies (1-bank tiles, finer pipelining)
COARSE_COPY = os.environ.get("CH2D_COARSE", "0") == "1"
VPS_BUFS = int(os.environ.get("CH2D_VPS", "2" if COARSE_COPY else "3"))
OPS_BUFS = int(os.environ.get("CH2D_OPS", "2" if COARSE_COPY else "5"))
LOADS_ON_GPSIMD = os.environ.get("CH2D_GPLOAD", "0") == "1"
WARMUP_MMS = int(os.environ.get("CH2D_WARMUP", "0"))


def _interp_matrix(n, u):
    """[n, Q] float64 matrix M with (y @ M) == cubic-Hermite interp of y at u,
    for grid x = arange(n), matching the reference's searchsorted/slope rules."""
    q = len(u)
    m = np.zeros((n, q), dtype=np.float64)
    idx = np.searchsorted(np.arange(1, n - 1, dtype=np.float64), u.astype(np.float64))
    t = u.astype(np.float64) - idx
    t2, t3 = t * t, t * t * t
    h00 = 1.0 - 3.0 * t2 + 2.0 * t3
    h10 = t - 2.0 * t2 + t3
    h01 = 3.0 * t2 - 2.0 * t3
    h11 = t3 - t2
    for k in range(q):
        i = int(idx[k])
        m[i, k] += h00[k]
        m[i + 1, k] += h01[k]
        if i == 0:
            m[1, k] += h10[k]
            m[0, k] -= h10[k]
        else:
            m[i + 1, k] += h10[k] / 2
            m[i - 1, k] -= h10[k] / 2
        if i + 1 == n - 1:
            m[n - 1, k] += h11[k]
            m[n - 2, k] -= h11[k]
        else:
            m[i + 2, k] += h11[k] / 2
            m[i, k] -= h11[k] / 2
    return m, idx.astype(np.int64)


def _make_groups(idx, n, max_size=512, bank=512):
    """Greedy contiguous query groups; each group's source rows fit a
    128-row window starting at row_lo.  Groups never cross `bank`-multiples
    in query index (PSUM bank boundary) and keep even sizes where possible
    (fp32r ISA needs even matmul N and 8B-aligned PSUM column offsets).
    Returns ([(q_start, q_end, row_lo)], f32r_ok)."""
    qn = len(idx)
    lo = np.maximum(idx - 1, 0)
    hi = np.minimum(idx + 2, n - 1)
    groups = []
    s = 0
    while s < qn:
        row_lo = int(lo[s])
        e = s
        while e < qn:
            if hi[e] - row_lo + 1 > P:
                break
            if e - s >= max_size:
                break
            if e > s and (e % bank) == 0:
                break
            e += 1
        if e < qn and (e - s) % 2 == 1 and e - s > 1:
            e -= 1  # keep sizes (and hence starts) even for fp32r
        groups.append((s, e, min(row_lo, n - P)))
        s = e
    f32r_ok = all(q % 2 == 0 and (e - q) % 2 == 0 for q, e, _ in groups)
    return groups, f32r_ok


def _build_nc(g1, g2, mm_dt):
    MM_DT = mm_dt
    nc = bacc.Bacc("TRN2", target_bir_lowering=False,
                   name="cubic_hermite2d", num_devices=N_CORES)
    sig_d = nc.dram_tensor("signal", [NB, H, W], MM_DT, kind="ExternalInput")
    w2_d = nc.dram_tensor("w2p", [P, NY], MM_DT, kind="ExternalInput")
    w1_d = nc.dram_tensor("w1p", [P, NX], MM_DT, kind="ExternalInput")
    out_d = nc.dram_tensor("out", [NB, NY, NX], F32, kind="ExternalOutput")

    wlo1_list = sorted({g[2] for g in g1})  # distinct xs source windows
    wlo2_list = sorted({g[2] for g in g2})  # distinct ys source windows
    copy_i = 0
    # per-bank halves so PSUM tiles are single-bank
    half1 = [[g for g in g1 if g[1] <= NX // 2], [g for g in g1 if g[0] >= NX // 2]]
    half2 = [[g for g in g2 if g[1] <= NY // 2], [g for g in g2 if g[0] >= NY // 2]]
    assert sum(map(len, half1)) == len(g1) and sum(map(len, half2)) == len(g2)

    with (
        TileContext(nc) as tc,
        tc.tile_pool(name="const", bufs=1) as const_pool,
        tc.tile_pool(name="sig", bufs=len(wlo2_list)) as sig_pool,
        tc.tile_pool(name="vbuf", bufs=NB * len(wlo1_list)) as v_pool,
        tc.tile_pool(name="obuf", bufs=8) as o_pool,
        tc.tile_pool(name="vps", bufs=VPS_BUFS, space="PSUM") as vps_pool,
        tc.tile_pool(name="ops", bufs=OPS_BUFS, space="PSUM") as ops_pool,
    ):
        load_eng = nc.gpsimd if LOADS_ON_GPSIMD else nc.sync
        # HAM warmup: the PE would otherwise idle at 1.2 GHz (K=4/8) through
        # the initial loads; ~3.4us of dummy matmuls flips it to 2.4 GHz so
        # the real stream starts warm.
        if WARMUP_MMS:
            warm = const_pool.tile([P, 512], MM_DT, name="warm")
            nc.vector.memset(warm[:], 0)
            wps = ops_pool.tile([P, NX], F32, name="ops")
            for i in range(WARMUP_MMS):
                nc.tensor.matmul(out=wps[:, :512], lhsT=warm[:, :P],
                                 rhs=warm[:, :512], start=True, stop=True)

        w2_s = const_pool.tile([P, NY], MM_DT, name="w2s")
        load_eng.dma_start(out=w2_s[:], in_=w2_d[:, :])

        def copy_out(dst, src):
            # alternate PSUM->SBUF copies between DVE and ACT to split the load
            nonlocal copy_i
            if copy_i % 2 == 0:
                nc.vector.tensor_copy(out=dst, in_=src)
            else:
                nc.scalar.copy(out=dst, in_=src)
            copy_i += 1

        # preload signal: per source window, batch 0 first (small, unblocks
        # the first matmuls fast), then batches 1..NB-1 in one strided DMA
        # dst [128, NB, W]; src (p, b, w) = signal[b, hlo + p, w]
        sig_tiles = {}
        for hlo in wlo2_list:
            st = sig_pool.tile([P, NB, W], MM_DT, name="sigt")
            load_eng.dma_start(out=st[:, 0, :], in_=sig_d[0, hlo:hlo + P, :])
            sig_tiles[hlo] = st
        w1_s = const_pool.tile([P, NX], MM_DT, name="w1s")
        load_eng.dma_start(out=w1_s[:], in_=w1_d[:, :])
        for hlo in wlo2_list:
            src = bass.AP(tensor=sig_d, offset=H * W + hlo * W,
                          ap=[[W, P], [H * W, NB - 1], [1, W]])
            load_eng.dma_start(out=sig_tiles[hlo][:, 1:, :], in_=src)

        def build_step1(b, v_tiles_all):
            v_tiles = {}
            for wlo in wlo1_list:
                vt = v_pool.tile([P, NY], MM_DT, name="vt")
                if COARSE_COPY:
                    vps = vps_pool.tile([P, NY], F32, name="vps")
                    for (rs, re, hlo) in g2:
                        nc.tensor.matmul(
                            out=vps[:, rs:re],
                            lhsT=sig_tiles[hlo][:, b, wlo:wlo + P],
                            rhs=w2_s[:, rs:re],
                            start=True, stop=True,
                        )
                    copy_out(vt[:], vps[:])
                else:
                    for hi_, hgroups in enumerate(half2):
                        if not hgroups:
                            continue
                        base = hi_ * (NY // 2)
                        vps = vps_pool.tile([P, NY // 2], F32, name="vps")
                        for (rs, re, hlo) in hgroups:
                            nc.tensor.matmul(
                                out=vps[:, rs - base:re - base],
                                lhsT=sig_tiles[hlo][:, b, wlo:wlo + P],
                                rhs=w2_s[:, rs:re],
                                start=True, stop=True,
                            )
                        copy_out(vt[:, base:base + NY // 2], vps[:])
                v_tiles[wlo] = vt
            v_tiles_all[b] = v_tiles

        def build_step2_block(b, mi_list, v_tiles):
            # one staging tile + one store covering r-blocks mi_list of batch b
            np_ = len(mi_list)
            ot = o_pool.tile([P, np_ * NX], F32, name="ot",
                             padded_shape=[P, 2 * NX])
            for sub, mi in enumerate(mi_list):
                if COARSE_COPY:
                    ops = ops_pool.tile([P, NX], F32, name="ops")
                    for (qs, qe, wlo) in g1:
                        nc.tensor.matmul(
                            out=ops[:, qs:qe],
                            lhsT=v_tiles[wlo][:, mi * P:(mi + 1) * P],
                            rhs=w1_s[:, qs:qe],
                            start=True, stop=True,
                        )
                    copy_out(ot[:, sub * NX:(sub + 1) * NX], ops[:])
                else:
                    for hi_, hgroups in enumerate(half1):
                        if not hgroups:
                            continue
                        base = hi_ * (NX // 2)
                        ops = ops_pool.tile([P, NX // 2], F32, name="ops")
                        for (qs, qe, wlo) in hgroups:
                            nc.tensor.matmul(
                                out=ops[:, qs - base:qe - base],
                                lhsT=v_tiles[wlo][:, mi * P:(mi + 1) * P],
                                rhs=w1_s[:, qs:qe],
                                start=True, stop=True,
                            )
                        copy_out(ot[:, sub * NX + base:sub * NX + base + NX // 2],
                                 ops[:])
            dst = bass.AP(tensor=out_d,
                          offset=b * NY * NX + mi_list[0] * P * NX,
                          ap=[[NX, P], [P * NX, np_], [1, NX]])
            nc.sync.dma_start(out=dst, in_=ot[:])

        v_all = {}
        # software pipeline at half-batch granularity: the next batch's
        # step1 (PE-heavy, store-free) is interleaved between the two
        # halves of the current batch's step2, smoothing store traffic.
        # The final batch stores per-block so the tail drains 512KB.
        build_step1(0, v_all)
        for b in range(NB):
            if b + 1 < NB:
                for mp in range(2):
                    build_step2_block(b, [2 * mp, 2 * mp + 1], v_all[b])
                build_step1(b + 1, v_all)
                for mp in range(2, 4):
                    build_step2_block(b, [2 * mp, 2 * mp + 1], v_all[b])
            else:
                for mi in range(NY // P):
                    build_step2_block(b, [mi], v_all[b])

    nc.compile()
    return nc


def _prepare(signal, x1, x2, xs, ys):
    """Host-side prep: sorted-order permutations, interp matrices, groups."""
    xs = np.asarray(xs, dtype=np.float32)
    ys = np.asarray(ys, dtype=np.float32)
    perm_x = None
    if np.any(np.diff(xs) < 0):
        perm_x = np.argsort(xs, kind="stable")
        xs = xs[perm_x]
    perm_y = None
    if np.any(np.diff(ys) < 0):
        perm_y = np.argsort(ys, kind="stable")
        ys = ys[perm_y]

    m1, i1 = _interp_matrix(W, xs)
    m2, i2 = _interp_matrix(H, ys)
    g1, ok1 = _make_groups(i1, W)
    g2, ok2 = _make_groups(i2, H)

    # pack band blocks: rows = the group's 128-row source window
    w1p = np.zeros((P, NX), dtype=np.float32)
    for (qs, qe, wlo) in g1:
        w1p[:, qs:qe] = m1[wlo:wlo + P, qs:qe]
    w2p = np.zeros((P, NY), dtype=np.float32)
    for (rs, re, hlo) in g2:
        w2p[:, rs:re] = m2[hlo:hlo + P, rs:re]
    return g1, g2, ok1 and ok2, w1p, w2p, perm_x, perm_y


_NC_CACHE = {}


def _run(inputs, trace=False, trace_kwargs=None):
    signal = np.ascontiguousarray(np.asarray(inputs["signal"], dtype=np.float32))
    g1, g2, f32r_ok, w1p, w2p, perm_x, perm_y = _prepare(
        signal, inputs["x1"], inputs["x2"], inputs["xs"], inputs["ys"])

    mode = MM_MODE
    if mode == "f32r" and not f32r_ok:
        mode = "f32"
    mm_dt = _MM_DTS[mode]
    key = (tuple(g1), tuple(g2), mm_dt)
    nc = _NC_CACHE.get(key)
    if nc is None:
        nc = _build_nc(g1, g2, mm_dt)
        _NC_CACHE[key] = nc

    np_dt = mybir.dt.np(mm_dt)
    sig_cast = signal.astype(np_dt) if np_dt != np.float32 else signal
    w1c, w2c = w1p.astype(np_dt), w2p.astype(np_dt)
    in_maps = []
    for c in range(N_CORES):
        in_maps.append({
            "signal": np.ascontiguousarray(sig_cast[c * NB:(c + 1) * NB]),
            "w2p": w2c,
            "w1p": w1c,
        })
    res = run_bass_kernel_spmd(
        nc, in_maps, core_ids=list(range(N_CORES)),
        trace=trace, **(trace_kwargs or {}),
    )
    out = np.concatenate([r["out"] for r in res.results], axis=0)

    # restore original (unsorted) query order if needed
    if perm_y is not None:
        inv = np.empty_like(perm_y)
        inv[perm_y] = np.arange(len(perm_y))
        out = out[:, inv, :]
    if perm_x is not None:
        inv = np.empty_like(perm_x)
        inv[perm_x] = np.arange(len(perm_x))
        out = out[:, :, inv]
    return out, res


def kernel(signal, x1, x2, xs, ys):
    out, _ = _run({"signal": signal, "x1": x1, "x2": x2, "xs": xs, "ys": ys})
    return out


# revision 42
# speedup vs baseline: 1.0656x; 1.0656x over previous
"""Trainium2 Bass kernel for 2D cubic Hermite interpolation (nn_CubicHermite2d).

Math: with x1 = arange(W), x2 = arange(H) (per the problem spec), the whole
op is linear in `signal`:

    result[b, r, q] = sum_{h,w} M2[h, r] * signal[b, h, w] * M1[w, q]

where M1 [W, Nx] / M2 [H, Ny] are 4-banded cubic-Hermite interpolation
matrices built on the host from xs / ys.  Queries are sorted, so greedy
contiguous query groups have source-row bands inside a single 128-row
window -> every output block is ONE K=128 matmul on the PE (no
accumulation, no transposes):

    step 1:  v[wlo][wp, r]  = sig[hlo:+128, wlo:+128].T @ M2[hlo:+128, rs:re]
    step 2:  out[b, rm, q]  = v[wlo][:, rm*128:+128].T @ M1[wlo:+128, qs:qe]

Matmuls default to float16: 1 cyc/row on the PE, FWL fast weight loads, and
half the load bytes; the inputs are O(1) randn so fp16 range is a non-issue
and the only cost is ~2^-12 input rounding (measured 9.1e-4 scale-relative
absmax vs the fp32 reference).  CH2D_DT=f32r/f32/bf16 selects other modes
(f32 is exact but 4 cyc/row).  Groups keep even sizes/starts so the f32r
mode also satisfies its even-N / 8B-aligned-PSUM ISA restrictions.

Each signal window loads for ALL batches in one 3D strided DMA; the build
software-pipelines step1(b+1) between the two step2 halves of batch b to
smooth store traffic, and the last batch stores per-r-block so the kernel
tail only drains a single 512KB store.

Sharding: data-parallel over batch B=32 across 8 cores (4 batches/core).
"""

import os
import sys

import numpy as np

for _p in ("/root/.axon_site", "/root/.axon_site/_ro/trn_rl_repo",
           "/root/.axon_site/_ro/pypackages", "/opt/trn_rl_repo"):
    if os.path.isdir(_p) and _p not in sys.path:
        sys.path.append(_p)

import concourse.bass as bass
import concourse.mybir as mybir
from concourse import bacc
from concourse.bass_utils import run_bass_kernel_spmd
from concourse.tile import TileContext

# Problem shapes (hardcoded per spec)
B, H, W = 32, 512, 512
NX, NY = 1024, 1024
N_CORES = 8
NB = B // N_CORES  # batches per core

P = 128
F32 = mybir.dt.float32
# matmul operand dtype: f16 (1 cyc/row, ~2^-12 input rounding, FWL weight
# loads, half DMA bytes) | f32r (2 cyc/row, ~2^-11 rounding) | f32 (4 cyc/row,
# exact) | bf16 (1 cyc/row, ~2^-9 rounding)
MM_MODE = os.environ.get("CH2D_DT", "f16")
_MM_DTS = {"f16": mybir.dt.float16, "bf16": mybir.dt.bfloat16,
           "f32r": mybir.dt.float32r, "f32": mybir.dt.float32}
# COARSE_COPY: one [128,1024] PSUM->SBUF copy per tile (2-bank PSUM tiles,
# fewer engine ops) vs two [128,512] copies (1-bank tiles, finer pipelining)
COARSE_COPY = os.environ.get("CH2D_COARSE", "0") == "1"
VPS_BUFS = int(os.environ.get("CH2D_VPS", "2" if COARSE_COPY else "3"))
OPS_BUFS = int(os.environ.get("CH2D_OPS", "2" if COARSE_COPY else "5"))
LOADS_ON_GPSIMD = os.environ.get("CH2D_GPLOAD", "0") == "1"
WARMUP_MMS = int(os.environ.get("CH2D_WARMUP", "0"))


def _interp_matrix(x0, u):
    """[n, Q] float64 matrix M with (y @ M) == _interp1d(y, x0, slopes, u) of
    the reference (searchsorted bucket, one-sided/averaged Hermite tangents).
    x0 is the sorted sample grid (the reference uses arange, but any sorted
    grid works here)."""
    x0 = np.asarray(x0, dtype=np.float64)
    n = len(x0)
    q = len(u)
    d = np.diff(x0)  # d[j] = x0[j+1] - x0[j]
    m = np.zeros((n, q), dtype=np.float64)
    idx = np.searchsorted(x0[1:-1], u.astype(np.float64))
    dxq = d[idx]
    t = (u.astype(np.float64) - x0[idx]) / dxq
    t2, t3 = t * t, t * t * t
    h00 = 1.0 - 3.0 * t2 + 2.0 * t3
    h10 = (t - 2.0 * t2 + t3) * dxq   # multiplies m[I]
    h01 = 3.0 * t2 - 2.0 * t3
    h11 = (t3 - t2) * dxq             # multiplies m[I+1]
    for k in range(q):
        i = int(idx[k])
        m[i, k] += h00[k]
        m[i + 1, k] += h01[k]
        c = h10[k]  # m[I]: one-sided at 0, averaged interior
        if i == 0:
            m[1, k] += c / d[0]
            m[0, k] -= c / d[0]
        else:
            m[i + 1, k] += 0.5 * c / d[i]
            m[i, k] += 0.5 * c * (1.0 / d[i - 1] - 1.0 / d[i])
            m[i - 1, k] -= 0.5 * c / d[i - 1]
        c = h11[k]  # m[I+1]
        if i + 1 == n - 1:
            m[n - 1, k] += c / d[n - 2]
            m[n - 2, k] -= c / d[n - 2]
        else:
            m[i + 2, k] += 0.5 * c / d[i + 1]
            m[i + 1, k] += 0.5 * c * (1.0 / d[i] - 1.0 / d[i + 1])
            m[i, k] -= 0.5 * c / d[i]
    return m, idx.astype(np.int64)


def _make_groups(idx, n, max_size=512, bank=512):
    """Greedy contiguous query groups; each group's source rows fit a
    128-row window starting at row_lo.  Groups never cross `bank`-multiples
    in query index (PSUM bank boundary) and keep even sizes where possible
    (fp32r ISA needs even matmul N and 8B-aligned PSUM column offsets).
    Returns ([(q_start, q_end, row_lo)], f32r_ok)."""
    qn = len(idx)
    lo = np.maximum(idx - 1, 0)
    hi = np.minimum(idx + 2, n - 1)
    groups = []
    s = 0
    while s < qn:
        row_lo = int(lo[s])
        e = s
        while e < qn:
            if hi[e] - row_lo + 1 > P:
                break
            if e - s >= max_size:
                break
            if e > s and (e % bank) == 0:
                break
            e += 1
        if e < qn and (e - s) % 2 == 1 and e - s > 1:
            e -= 1  # keep sizes (and hence starts) even for fp32r
        groups.append((s, e, min(row_lo, n - P)))
        s = e
    f32r_ok = all(q % 2 == 0 and (e - q) % 2 == 0 for q, e, _ in groups)
    return groups, f32r_ok


def _build_nc(g1, g2, mm_dt):
    MM_DT = mm_dt
    nc = bacc.Bacc("TRN2", target_bir_lowering=False,
                   name="cubic_hermite2d", num_devices=N_CORES)
    sig_d = nc.dram_tensor("signal", [NB, H, W], MM_DT, kind="ExternalInput")
    w2_d = nc.dram_tensor("w2p", [P, NY], MM_DT, kind="ExternalInput")
    w1_d = nc.dram_tensor("w1p", [P, NX], MM_DT, kind="ExternalInput")
    out_d = nc.dram_tensor("out", [NB, NY, NX], F32, kind="ExternalOutput")

    wlo1_list = sorted({g[2] for g in g1})  # distinct xs source windows
    wlo2_list = sorted({g[2] for g in g2})  # distinct ys source windows
    copy_i = 0
    # per-bank halves so PSUM tiles are single-bank
    half1 = [[g for g in g1 if g[1] <= NX // 2], [g for g in g1 if g[0] >= NX // 2]]
    half2 = [[g for g in g2 if g[1] <= NY // 2], [g for g in g2 if g[0] >= NY // 2]]
    assert sum(map(len, half1)) == len(g1) and sum(map(len, half2)) == len(g2)

    with (
        TileContext(nc) as tc,
        tc.tile_pool(name="const", bufs=1) as const_pool,
        tc.tile_pool(name="sig", bufs=len(wlo2_list)) as sig_pool,
        tc.tile_pool(name="vbuf", bufs=NB * len(wlo1_list)) as v_pool,
        tc.tile_pool(name="obuf", bufs=8) as o_pool,
        tc.tile_pool(name="vps", bufs=VPS_BUFS, space="PSUM") as vps_pool,
        tc.tile_pool(name="ops", bufs=OPS_BUFS, space="PSUM") as ops_pool,
    ):
        load_eng = nc.gpsimd if LOADS_ON_GPSIMD else nc.sync
        # HAM warmup: the PE would otherwise idle at 1.2 GHz (K=4/8) through
        # the initial loads; ~3.4us of dummy matmuls flips it to 2.4 GHz so
        # the real stream starts warm.
        if WARMUP_MMS:
            warm = const_pool.tile([P, 512], MM_DT, name="warm")
            nc.vector.memset(warm[:], 0)
            wps = ops_pool.tile([P, NX], F32, name="ops")
            for i in range(WARMUP_MMS):
                nc.tensor.matmul(out=wps[:, :512], lhsT=warm[:, :P],
                                 rhs=warm[:, :512], start=True, stop=True)

        w2_s = const_pool.tile([P, NY], MM_DT, name="w2s")
        load_eng.dma_start(out=w2_s[:], in_=w2_d[:, :])

        def copy_out(dst, src):
            # alternate PSUM->SBUF copies between DVE and ACT to split the load
            nonlocal copy_i
            if copy_i % 2 == 0:
                nc.vector.tensor_copy(out=dst, in_=src)
            else:
                nc.scalar.copy(out=dst, in_=src)
            copy_i += 1

        # preload signal: per source window, batch 0 first (small, unblocks
        # the first matmuls fast), then batches 1..NB-1 in one strided DMA
        # dst [128, NB, W]; src (p, b, w) = signal[b, hlo + p, w]
        sig_tiles = {}
        for hlo in wlo2_list:
            st = sig_pool.tile([P, NB, W], MM_DT, name="sigt")
            load_eng.dma_start(out=st[:, 0, :], in_=sig_d[0, hlo:hlo + P, :])
            sig_tiles[hlo] = st
        w1_s = const_pool.tile([P, NX], MM_DT, name="w1s")
        load_eng.dma_start(out=w1_s[:], in_=w1_d[:, :])
        for hlo in wlo2_list:
            src = bass.AP(tensor=sig_d, offset=H * W + hlo * W,
                          ap=[[W, P], [H * W, NB - 1], [1, W]])
            load_eng.dma_start(out=sig_tiles[hlo][:, 1:, :], in_=src)

        def build_step1(b, v_tiles_all):
            v_tiles = {}
            for wlo in wlo1_list:
                vt = v_pool.tile([P, NY], MM_DT, name="vt")
                if COARSE_COPY:
                    vps = vps_pool.tile([P, NY], F32, name="vps")
                    for (rs, re, hlo) in g2:
                        nc.tensor.matmul(
                            out=vps[:, rs:re],
                            lhsT=sig_tiles[hlo][:, b, wlo:wlo + P],
                            rhs=w2_s[:, rs:re],
                            start=True, stop=True,
                        )
                    copy_out(vt[:], vps[:])
                else:
                    for hi_, hgroups in enumerate(half2):
                        if not hgroups:
                            continue
                        base = hi_ * (NY // 2)
                        vps = vps_pool.tile([P, NY // 2], F32, name="vps")
                        for (rs, re, hlo) in hgroups:
                            nc.tensor.matmul(
                                out=vps[:, rs - base:re - base],
                                lhsT=sig_tiles[hlo][:, b, wlo:wlo + P],
                                rhs=w2_s[:, rs:re],
                                start=True, stop=True,
                            )
                        copy_out(vt[:, base:base + NY // 2], vps[:])
                v_tiles[wlo] = vt
            v_tiles_all[b] = v_tiles

        def build_step2_block(b, mi_list, v_tiles):
            # one staging tile + one store covering r-blocks mi_list of batch b
            np_ = len(mi_list)
            ot = o_pool.tile([P, np_ * NX], F32, name="ot",
                             padded_shape=[P, 2 * NX])
            for sub, mi in enumerate(mi_list):
                if COARSE_COPY:
                    ops = ops_pool.tile([P, NX], F32, name="ops")
                    for (qs, qe, wlo) in g1:
                        nc.tensor.matmul(
                            out=ops[:, qs:qe],
                            lhsT=v_tiles[wlo][:, mi * P:(mi + 1) * P],
                            rhs=w1_s[:, qs:qe],
                            start=True, stop=True,
                        )
                    copy_out(ot[:, sub * NX:(sub + 1) * NX], ops[:])
                else:
                    for hi_, hgroups in enumerate(half1):
                        if not hgroups:
                            continue
                        base = hi_ * (NX // 2)
                        ops = ops_pool.tile([P, NX // 2], F32, name="ops")
                        for (qs, qe, wlo) in hgroups:
                            nc.tensor.matmul(
                                out=ops[:, qs - base:qe - base],
                                lhsT=v_tiles[wlo][:, mi * P:(mi + 1) * P],
                                rhs=w1_s[:, qs:qe],
                                start=True, stop=True,
                            )
                        copy_out(ot[:, sub * NX + base:sub * NX + base + NX // 2],
                                 ops[:])
            dst = bass.AP(tensor=out_d,
                          offset=b * NY * NX + mi_list[0] * P * NX,
                          ap=[[NX, P], [P * NX, np_], [1, NX]])
            nc.sync.dma_start(out=dst, in_=ot[:])

        v_all = {}
        # software pipeline at half-batch granularity: the next batch's
        # step1 (PE-heavy, store-free) is interleaved between the two
        # halves of the current batch's step2, smoothing store traffic.
        # The final batch stores per-block so the tail drains 512KB.
        build_step1(0, v_all)
        for b in range(NB):
            if b + 1 < NB:
                for mp in range(2):
                    build_step2_block(b, [2 * mp, 2 * mp + 1], v_all[b])
                build_step1(b + 1, v_all)
                for mp in range(2, 4):
                    build_step2_block(b, [2 * mp, 2 * mp + 1], v_all[b])
            else:
                for mi in range(NY // P):
                    build_step2_block(b, [mi], v_all[b])

    nc.compile()
    return nc


def _prepare(signal, x1, x2, xs, ys):
    """Host-side prep: sorted-order permutations, interp matrices, groups."""
    xs = np.asarray(xs, dtype=np.float32)
    ys = np.asarray(ys, dtype=np.float32)
    perm_x = None
    if np.any(np.diff(xs) < 0):
        perm_x = np.argsort(xs, kind="stable")
        xs = xs[perm_x]
    perm_y = None
    if np.any(np.diff(ys) < 0):
        perm_y = np.argsort(ys, kind="stable")
        ys = ys[perm_y]

    m1, i1 = _interp_matrix(np.asarray(x1, dtype=np.float64), xs)
    m2, i2 = _interp_matrix(np.asarray(x2, dtype=np.float64), ys)
    g1, ok1 = _make_groups(i1, W)
    g2, ok2 = _make_groups(i2, H)

    # pack band blocks: rows = the group's 128-row source window
    w1p = np.zeros((P, NX), dtype=np.float32)
    for (qs, qe, wlo) in g1:
        w1p[:, qs:qe] = m1[wlo:wlo + P, qs:qe]
    w2p = np.zeros((P, NY), dtype=np.float32)
    for (rs, re, hlo) in g2:
        w2p[:, rs:re] = m2[hlo:hlo + P, rs:re]
    return g1, g2, ok1 and ok2, w1p, w2p, perm_x, perm_y


_NC_CACHE = {}


def _run(inputs, trace=False, trace_kwargs=None):
    signal = np.ascontiguousarray(np.asarray(inputs["signal"], dtype=np.float32))
    g1, g2, f32r_ok, w1p, w2p, perm_x, perm_y = _prepare(
        signal, inputs["x1"], inputs["x2"], inputs["xs"], inputs["ys"])

    mode = MM_MODE
    if mode == "f32r" and not f32r_ok:
        mode = "f32"
    mm_dt = _MM_DTS[mode]
    key = (tuple(g1), tuple(g2), mm_dt)
    nc = _NC_CACHE.get(key)
    if nc is None:
        nc = _build_nc(g1, g2, mm_dt)
        _NC_CACHE[key] = nc

    np_dt = mybir.dt.np(mm_dt)
    sig_cast = signal.astype(np_dt) if np_dt != np.float32 else signal
    w1c, w2c = w1p.astype(np_dt), w2p.astype(np_dt)
    in_maps = []
    for c in range(N_CORES):
        in_maps.append({
            "signal": np.ascontiguousarray(sig_cast[c * NB:(c + 1) * NB]),
            "w2p": w2c,
            "w1p": w1c,
        })
    res = run_bass_kernel_spmd(
        nc, in_maps, core_ids=list(range(N_CORES)),
        trace=trace, **(trace_kwargs or {}),
    )
    out = np.concatenate([r["out"] for r in res.results], axis=0)

    # restore original (unsorted) query order if needed
    if perm_y is not None:
        inv = np.empty_like(perm_y)
        inv[perm_y] = np.arange(len(perm_y))
        out = out[:, inv, :]
    if perm_x is not None:
        inv = np.empty_like(perm_x)
        inv[perm_x] = np.arange(len(perm_x))
        out = out[:, :, inv]
    return out, res


def kernel(signal, x1, x2, xs, ys):
    out, _ = _run({"signal": signal, "x1": x1, "x2": x2, "xs": xs, "ys": ys})
    return out
